# revision 10
# baseline (speedup 1.0000x reference)
"""TRN2 Bass kernel for DenseDilatedKnnGraph (B=4, C=64, N=4096, k=9, dilation=2).

Algorithm
---------
reference: xt (B,N,C); dist(i,j) = |xi|^2 - 2<xi,xj> + |xj|^2; nn_idx = top-18
of -dist per row; output nn_idx[..., ::2] plus a center-index row
-> (2, B, N, 9) int32.

Per-row ordering of -dist equals the ordering of s_ij = 2<xi,xj> - |xj|^2.

Device (per core, SPMD over 8 cores; core = (batch, query-half)):
  - S via ONE fp8e4 DoubleRow matmul per 512-column chunk (0.5 cyc/col on the
    PE -- immune to the mid-pstate clock the PE is stuck at when its duty
    cycle is low).  K = 2 interleaved k-tiles of 128:
      ktile0 rows 0..63: qa x ca        rows 64..127: qb x ca
      ktile1 rows 0..63: qa x cb        rows 64..123: qb[0:60] x cb[0:60]
             rows 124..127: 1.0 x s1..s4 (4-level fp8 split of -|ca+cb|^2)
    where qa/qb = 2-level fp8e4 split of 2*x_q, ca/cb of x_cand.  S~ error
    sigma ~2.4e-2, max ~0.2 -- far below the value gaps at the family-select
    margin (measured worst needed-slot rank 19 of 2048).  Selection is
    approximate; the final ranking is exact (host, below).
  - PSUM evacuation fused with the select fold.  Only Act and DVE can read
    PSUM (DMA has no route, GpSimd has no port, DVE has ONE psum operand per
    instruction), so:
      Act: a32 = fp32 SBUF copy of PSUM halves PA1 (cols 0:1024) and
           PA2 (cols 2048:3072)                       -- 2 insts/tile
      DVE: u16[:, :1024]  = max(PM1_psum, a32[:, :1024])  -> fp16
           u16[:, 1024:]  = max(PM2_psum, a32[:, 1024:])  -> fp16
           (PM1 = cols 3072:4096, PM2 = cols 1024:2048)   -- 2 insts/tile
    Each DVE instruction evacuates 1024 PSUM values AND folds 2:1.
    Family slots (2048/row): slot c<1024: {c, 3072+c};
    slot 1024+u: {1024+u, 2048+u}.  No on-device top-k at all (the old
    kernel's max8/max_index full-row scans made DVE the 175us bottleneck).
  - DMA out u16 (2048 x 2048 fp16 per core).

Host: a family holding a true top-18 value ranks <= ~19 among the 2048 slot
values (at most 17 strictly larger values exist + fp8/fp16 noise), so the
top-40 slots per row are a guaranteed superset.  Gather their 80 candidate
columns, recompute s exactly (float64), stable top-18, dilate ::2.

History: 176.3us (max8/max_index full-row DVE scans) -> 104.0us (fp16 1-mm +
fold tree on DVE/Act) -> this version.
"""

import numpy as np
import ml_dtypes

import concourse.bacc as bacc
import concourse.mybir as mybir
import concourse.tile as tile
from concourse.bass_utils import run_bass_kernel_spmd

# Problem constants (hardcoded per harness contract).
B = 4
C = 64
N = 4096
K = 9
DILATION = 2
K_EFF = K * DILATION      # 18
P = 128                   # partitions / queries per tile
KM = 128                  # matmul contraction rows per k-tile
N_CORES = 8
QROWS = (B * N) // N_CORES          # 2048 query rows per core
N_TILES = QROWS // P                # 16 tiles per core
NSLOT = 2048              # family slots per row
KSEL = 40                 # slots resolved exactly on host
NQL = 60                  # qb channels kept in ktile1 (124..127 are bias)
F8 = ml_dtypes.float8_e4m3


def _build_program(n_tiles=N_TILES):
    nc = bacc.Bacc(
        "TRN2", target_bir_lowering=False, debug=False, enable_asserts=False
    )
    f32 = mybir.dt.float32
    f16 = mybir.dt.float16
    f8 = mybir.dt.float8e4
    amax = mybir.AluOpType.max
    dr = mybir.MatmulPerfMode.DoubleRow
    nq = n_tiles * P
    lhs = nc.dram_tensor("lhs", (KM, 2, nq), f8, kind="ExternalInput")
    rhs = nc.dram_tensor("rhs", (KM, 2, N), f8, kind="ExternalInput")
    u_out = nc.dram_tensor("u_out", (nq, NSLOT), f16, kind="ExternalOutput")
    lhs_ap, rhs_ap, u_ap = lhs.ap(), rhs.ap(), u_out.ap()

    with tile.TileContext(nc) as tc:
        with (
            tc.tile_pool(name="const", bufs=1) as cpool,
            tc.tile_pool(name="psA", bufs=1, space="PSUM") as ppoolA,
            tc.tile_pool(name="psB", bufs=1, space="PSUM") as ppoolB,
            tc.tile_pool(name="work", bufs=2) as wpool,
            tc.tile_pool(name="outp", bufs=4) as opool,
        ):
            # dependency-free warm-up matmuls during the input-DMA prologue
            prime = cpool.tile([KM, 2, 512], f8)
            nc.gpsimd.memset(prime[:, :, :], 0.0)
            warm = ppoolA.tile([P, 2048], f32, tag="pa")
            for _ in range(12):
                nc.tensor.matmul(warm[:, :512], prime[:, :, :128],
                                 prime[:, :, :], start=True, stop=True,
                                 perf_mode=dr)

            # chunked input loads so the first matmul only waits for 128KB
            rh_sb = [
                cpool.tile([KM, 2, 512], f8, name=f"rh{j}", tag=f"rh{j}")
                for j in range(8)
            ]
            lh_sb = [
                cpool.tile([KM, 2, 512], f8, name=f"lh{i}", tag=f"lh{i}")
                for i in range(4)
            ]
            nc.sync.dma_start(rh_sb[0][:, :, :], rhs_ap[:, :, 0:512])
            nc.sync.dma_start(lh_sb[0][:, :, :], lhs_ap[:, :, 0:512])
            nc.sync.dma_start(rh_sb[1][:, :, :], rhs_ap[:, :, 512:1024])
            for j in range(2, 8):
                nc.sync.dma_start(rh_sb[j][:, :, :],
                                  rhs_ap[:, :, j * 512 : (j + 1) * 512])
            for i in range(1, 4):
                nc.sync.dma_start(lh_sb[i][:, :, :],
                                  lhs_ap[:, :, i * 512 : (i + 1) * 512])

            for t in range(n_tiles):
                lsl = lh_sb[t // 4][:, :, (t % 4) * P : (t % 4 + 1) * P]
                pa = ppoolA.tile([P, 2048], f32, tag="pa")    # cols 0:1024, 2048:3072
                pm1 = ppoolB.tile([P, 1024], f32, tag="pm1")  # cols 3072:4096
                pm2 = ppoolB.tile([P, 1024], f32, tag="pm2")  # cols 1024:2048
                for i, j in enumerate((0, 1, 4, 5)):
                    nc.tensor.matmul(
                        pa[:, i * 512 : (i + 1) * 512],
                        lsl, rh_sb[j][:, :, :],
                        start=True, stop=True, perf_mode=dr,
                    )
                for ps, j0 in ((pm1, 6), (pm2, 2)):
                    for h in range(2):
                        nc.tensor.matmul(
                            ps[:, h * 512 : (h + 1) * 512],
                            lsl, rh_sb[j0 + h][:, :, :],
                            start=True, stop=True, perf_mode=dr,
                        )

                a32 = wpool.tile([P, 2048], f32, tag="a32")
                nc.scalar.copy(a32[:, :], pa[:, :])

                u16 = opool.tile([P, NSLOT], f16, tag="u16")
                nc.vector.tensor_tensor(
                    u16[:, :1024], pm1[:, :], a32[:, :1024], amax)
                nc.vector.tensor_tensor(
                    u16[:, 1024:], pm2[:, :], a32[:, 1024:], amax)

                nc.sync.dma_start(u_ap[t * P : (t + 1) * P, :], u16[:])
    nc.compile()
    return nc


def _split8(a):
    hi = a.astype(F8)
    lo = (a - hi.astype(np.float32)).astype(F8)
    return hi, lo


def _prep_core_inputs(X, core):
    """X: (B, N, C) fp32. Returns input map for one core."""
    b, h = divmod(core, N_CORES // B)
    Xb = X[b]
    ca, cb = _split8(Xb.T)                             # (C, N) each
    ceff = ca.astype(np.float32) + cb.astype(np.float32)
    csq = np.einsum("cn,cn->n", ceff, ceff)
    r = -csq
    svec = np.zeros((4, N), F8)
    for lv in range(4):
        svec[lv] = r.astype(F8)
        r = r - svec[lv].astype(np.float32)
    rhs = np.zeros((KM, 2, N), F8)
    rhs[:C, 0] = ca
    rhs[C:, 0] = ca
    rhs[:C, 1] = cb
    rhs[C : C + NQL, 1] = cb[:NQL]
    rhs[C + NQL :, 1] = svec

    Q = 2.0 * Xb[h * QROWS : (h + 1) * QROWS]          # (QROWS, C)
    qa, qb = _split8(Q.T)                              # (C, QROWS)
    lhs = np.zeros((KM, 2, QROWS), F8)
    lhs[:C, 0] = qa
    lhs[C:, 0] = qb
    lhs[:C, 1] = qa
    lhs[C : C + NQL, 1] = qb[:NQL]
    lhs[C + NQL :, 1] = 1.0
    return {"lhs": lhs, "rhs": rhs}


def _slot_map():
    """slot -> 2 candidate columns (see fold structure in the header)."""
    m = np.empty((NSLOT, 2), np.int64)
    c = np.arange(1024)
    m[:1024, 0] = c
    m[:1024, 1] = 3072 + c
    m[1024:, 0] = 1024 + c
    m[1024:, 1] = 2048 + c
    return m


_NC_CACHE = {}
_SLOT_MAP = _slot_map()


def kernel(x: np.ndarray) -> np.ndarray:
    x = np.asarray(x)
    assert x.shape == (B, C, N, 1), x.shape
    X = np.ascontiguousarray(np.transpose(x[..., 0], (0, 2, 1)))  # (B, N, C)

    if N_TILES not in _NC_CACHE:
        _NC_CACHE[N_TILES] = _build_program(N_TILES)
    nc = _NC_CACHE[N_TILES]

    in_maps = [_prep_core_inputs(X, c) for c in range(N_CORES)]
    res = run_bass_kernel_spmd(nc, in_maps, core_ids=list(range(N_CORES)))

    nn_idx = np.empty((B, N, K_EFF), np.int64)
    for core in range(N_CORES):
        b, h = divmod(core, N_CORES // B)
        U = np.asarray(res.results[core]["u_out"])        # (QROWS, NSLOT) f16
        sel = np.argpartition(-U, KSEL, axis=1)[:, :KSEL]
        cand = _SLOT_MAP[sel].reshape(QROWS, 2 * KSEL)
        cand.sort(axis=1)
        Xb = X[b].astype(np.float64)
        Q = 2.0 * Xb[h * QROWS : (h + 1) * QROWS]
        xsq = np.einsum("nc,nc->n", Xb, Xb)
        G = Xb[cand]                                      # (QROWS, 2K, C)
        s_c = np.einsum("rkc,rc->rk", G, Q) - xsq[cand]
        oo = np.argsort(-s_c, axis=1, kind="stable")[:, :K_EFF]
        nn_idx[b, h * QROWS : (h + 1) * QROWS] = np.take_along_axis(
            cand, oo, axis=1
        )

    nn_dil = nn_idx[:, :, ::DILATION]                     # (B, N, 9)
    center = np.broadcast_to(np.arange(N)[None, :, None], nn_dil.shape)
    out = np.stack((nn_dil, center), axis=0).astype(np.int32)
    return out


# revision 12
# speedup vs baseline: 1.2038x; 1.2038x over previous
"""TRN2 Bass kernel for DenseDilatedKnnGraph (B=4, C=64, N=4096, k=9, dilation=2).

Algorithm
---------
reference: xt (B,N,C); dist(i,j) = |xi|^2 - 2<xi,xj> + |xj|^2; nn_idx = top-18
of -dist per row; output nn_idx[..., ::2] plus a center-index row
-> (2, B, N, 9) int32.

Per-row ordering of -dist equals the ordering of s_ij = 2<xi,xj> - |xj|^2.

Device (per core, SPMD over 8 cores; core = (batch, query-half)):
  - S via ONE fp8e4 DoubleRow matmul per 512-column chunk (0.5 cyc/col on the
    PE -- immune to the mid-pstate clock the PE is stuck at when its duty
    cycle is low).  K = 2 interleaved k-tiles of 128:
      ktile0 rows 0..63: qa x ca        rows 64..127: qb x ca
      ktile1 rows 0..63: qa x cb        rows 64..123: qb[0:60] x cb[0:60]
             rows 124..127: 1.0 x s1..s4 (4-level fp8 split of -|ca+cb|^2)
    where qa/qb = 2-level fp8e4 split of 2*x_q, ca/cb of x_cand.  S~ error
    sigma ~2.4e-2, max ~0.2 -- far below the value gaps at the family-select
    margin (measured worst needed-slot rank 19 of 2048).  Selection is
    approximate; the final ranking is exact (host, below).
  - PSUM evacuation fused with the select fold.  Only Act and DVE can read
    PSUM (DMA has no route, GpSimd has no port, DVE has ONE psum operand per
    instruction), so:
      Act: a32 = fp32 SBUF copy of PSUM halves PA1 (cols 0:1024) and
           PA2 (cols 2048:3072)                       -- 2 insts/tile
      DVE: u16[:, :1024]  = max(PM1_psum, a32[:, :1024])  -> fp16
           u16[:, 1024:]  = max(PM2_psum, a32[:, 1024:])  -> fp16
           (PM1 = cols 3072:4096, PM2 = cols 1024:2048)   -- 2 insts/tile
    Each DVE instruction evacuates 1024 PSUM values AND folds 2:1.
    Family slots (2048/row): slot c<1024: {c, 3072+c};
    slot 1024+u: {1024+u, 2048+u}.  No on-device top-k at all (the old
    kernel's max8/max_index full-row scans made DVE the 175us bottleneck).
  - DMA out u16 (2048 x 2048 fp16 per core).

Host: a family holding a true top-18 value ranks <= ~19 among the 2048 slot
values (at most 17 strictly larger values exist + fp8/fp16 noise), so the
top-40 slots per row are a guaranteed superset.  Gather their 80 candidate
columns, recompute s exactly (float64), stable top-18, dilate ::2.

History: 176.3us (max8/max_index full-row DVE scans) -> 104.0us (fp16 1-mm +
fold tree on DVE/Act) -> this version.
"""

import numpy as np
import ml_dtypes

import concourse.bacc as bacc
import concourse.mybir as mybir
import concourse.tile as tile
from concourse.bass_utils import run_bass_kernel_spmd

# Problem constants (hardcoded per harness contract).
B = 4
C = 64
N = 4096
K = 9
DILATION = 2
K_EFF = K * DILATION      # 18
P = 128                   # partitions / queries per tile
KM = 128                  # matmul contraction rows per k-tile
N_CORES = 8
QROWS = (B * N) // N_CORES          # 2048 query rows per core
N_TILES = QROWS // P                # 16 tiles per core
NSLOT = 2048              # family slots per row
KSEL = 40                 # slots resolved exactly on host
NQL = 60                  # qb channels kept in ktile1 (124..127 are bias)
F8 = ml_dtypes.float8_e4m3


def _build_program(n_tiles=N_TILES):
    nc = bacc.Bacc(
        "TRN2", target_bir_lowering=False, debug=False, enable_asserts=False
    )
    f32 = mybir.dt.float32
    f16 = mybir.dt.float16
    f8 = mybir.dt.float8e4
    amax = mybir.AluOpType.max
    dr = mybir.MatmulPerfMode.DoubleRow
    nq = n_tiles * P
    lhs = nc.dram_tensor("lhs", (KM, 2, nq), f8, kind="ExternalInput")
    rhs = nc.dram_tensor("rhs", (KM, 2, N), f8, kind="ExternalInput")
    u_out = nc.dram_tensor("u_out", (nq, NSLOT), f16, kind="ExternalOutput")
    lhs_ap, rhs_ap, u_ap = lhs.ap(), rhs.ap(), u_out.ap()

    with tile.TileContext(nc) as tc:
        with (
            tc.tile_pool(name="const", bufs=1) as cpool,
            tc.tile_pool(name="psA", bufs=1, space="PSUM") as ppoolA,
            tc.tile_pool(name="psB", bufs=1, space="PSUM") as ppoolB,
            tc.tile_pool(name="work", bufs=2) as wpool,
            tc.tile_pool(name="outp", bufs=4) as opool,
        ):
            # dependency-free warm-up matmuls during the input-DMA prologue
            prime = cpool.tile([KM, 2, 512], f8)
            nc.gpsimd.memset(prime[:, :, :], 0.0)
            warm = ppoolA.tile([P, 1024], f32, tag="pa1")
            for _ in range(12):
                nc.tensor.matmul(warm[:, :512], prime[:, :, :128],
                                 prime[:, :, :], start=True, stop=True,
                                 perf_mode=dr)

            # chunked input loads so the first matmul only waits for 128KB
            rh_sb = [
                cpool.tile([KM, 2, 512], f8, name=f"rh{j}", tag=f"rh{j}")
                for j in range(8)
            ]
            lh_sb = [
                cpool.tile([KM, 2, 512], f8, name=f"lh{i}", tag=f"lh{i}")
                for i in range(4)
            ]
            nc.sync.dma_start(rh_sb[0][:, :, :], rhs_ap[:, :, 0:512])
            nc.sync.dma_start(lh_sb[0][:, :, :], lhs_ap[:, :, 0:512])
            nc.sync.dma_start(rh_sb[1][:, :, :], rhs_ap[:, :, 512:1024])
            for j in range(2, 8):
                nc.sync.dma_start(rh_sb[j][:, :, :],
                                  rhs_ap[:, :, j * 512 : (j + 1) * 512])
            for i in range(1, 4):
                nc.sync.dma_start(lh_sb[i][:, :, :],
                                  lhs_ap[:, :, i * 512 : (i + 1) * 512])

            for t in range(n_tiles):
                lsl = lh_sb[t // 4][:, :, (t % 4) * P : (t % 4 + 1) * P]
                pa1 = ppoolA.tile([P, 1024], f32, tag="pa1")  # cols 0:1024
                pa2 = ppoolA.tile([P, 1024], f32, tag="pa2")  # cols 2048:3072
                pm1 = ppoolB.tile([P, 1024], f32, tag="pm1")  # cols 3072:4096
                pm2 = ppoolB.tile([P, 1024], f32, tag="pm2")  # cols 1024:2048
                for ps, j0 in ((pa1, 0), (pa2, 4), (pm1, 6), (pm2, 2)):
                    for h in range(2):
                        nc.tensor.matmul(
                            ps[:, h * 512 : (h + 1) * 512],
                            lsl, rh_sb[j0 + h][:, :, :],
                            start=True, stop=True, perf_mode=dr,
                        )

                a32a = wpool.tile([P, 1024], f32, tag="a32a")
                a32b = wpool.tile([P, 1024], f32, tag="a32b")
                nc.scalar.copy(a32a[:, :], pa1[:, :])
                nc.scalar.copy(a32b[:, :], pa2[:, :])

                u16 = opool.tile([P, NSLOT], f16, tag="u16")
                nc.vector.tensor_tensor(
                    u16[:, :1024], pm1[:, :], a32a[:, :], amax)
                nc.vector.tensor_tensor(
                    u16[:, 1024:], pm2[:, :], a32b[:, :], amax)

                nc.sync.dma_start(u_ap[t * P : (t + 1) * P, :], u16[:])
    nc.compile()
    return nc


def _split8(a):
    hi = a.astype(F8)
    lo = (a - hi.astype(np.float32)).astype(F8)
    return hi, lo


def _prep_core_inputs(X, core):
    """X: (B, N, C) fp32. Returns input map for one core."""
    b, h = divmod(core, N_CORES // B)
    Xb = X[b]
    ca, cb = _split8(Xb.T)                             # (C, N) each
    ceff = ca.astype(np.float32) + cb.astype(np.float32)
    csq = np.einsum("cn,cn->n", ceff, ceff)
    r = -csq
    svec = np.zeros((4, N), F8)
    for lv in range(4):
        svec[lv] = r.astype(F8)
        r = r - svec[lv].astype(np.float32)
    rhs = np.zeros((KM, 2, N), F8)
    rhs[:C, 0] = ca
    rhs[C:, 0] = ca
    rhs[:C, 1] = cb
    rhs[C : C + NQL, 1] = cb[:NQL]
    rhs[C + NQL :, 1] = svec

    Q = 2.0 * Xb[h * QROWS : (h + 1) * QROWS]          # (QROWS, C)
    qa, qb = _split8(Q.T)                              # (C, QROWS)
    lhs = np.zeros((KM, 2, QROWS), F8)
    lhs[:C, 0] = qa
    lhs[C:, 0] = qb
    lhs[:C, 1] = qa
    lhs[C : C + NQL, 1] = qb[:NQL]
    lhs[C + NQL :, 1] = 1.0
    return {"lhs": lhs, "rhs": rhs}


def _slot_map():
    """slot -> 2 candidate columns (see fold structure in the header)."""
    m = np.empty((NSLOT, 2), np.int64)
    c = np.arange(1024)
    m[:1024, 0] = c
    m[:1024, 1] = 3072 + c
    m[1024:, 0] = 1024 + c
    m[1024:, 1] = 2048 + c
    return m


_NC_CACHE = {}
_SLOT_MAP = _slot_map()


def kernel(x: np.ndarray) -> np.ndarray:
    x = np.asarray(x)
    assert x.shape == (B, C, N, 1), x.shape
    X = np.ascontiguousarray(np.transpose(x[..., 0], (0, 2, 1)))  # (B, N, C)

    if N_TILES not in _NC_CACHE:
        _NC_CACHE[N_TILES] = _build_program(N_TILES)
    nc = _NC_CACHE[N_TILES]

    in_maps = [_prep_core_inputs(X, c) for c in range(N_CORES)]
    res = run_bass_kernel_spmd(nc, in_maps, core_ids=list(range(N_CORES)))

    nn_idx = np.empty((B, N, K_EFF), np.int64)
    for core in range(N_CORES):
        b, h = divmod(core, N_CORES // B)
        U = np.asarray(res.results[core]["u_out"])        # (QROWS, NSLOT) f16
        sel = np.argpartition(-U, KSEL, axis=1)[:, :KSEL]
        cand = _SLOT_MAP[sel].reshape(QROWS, 2 * KSEL)
        cand.sort(axis=1)
        Xb = X[b].astype(np.float64)
        Q = 2.0 * Xb[h * QROWS : (h + 1) * QROWS]
        xsq = np.einsum("nc,nc->n", Xb, Xb)
        G = Xb[cand]                                      # (QROWS, 2K, C)
        s_c = np.einsum("rkc,rc->rk", G, Q) - xsq[cand]
        oo = np.argsort(-s_c, axis=1, kind="stable")[:, :K_EFF]
        nn_idx[b, h * QROWS : (h + 1) * QROWS] = np.take_along_axis(
            cand, oo, axis=1
        )

    nn_dil = nn_idx[:, :, ::DILATION]                     # (B, N, 9)
    center = np.broadcast_to(np.arange(N)[None, :, None], nn_dil.shape)
    out = np.stack((nn_dil, center), axis=0).astype(np.int32)
    return out


# revision 16
# speedup vs baseline: 1.2446x; 1.0340x over previous
"""TRN2 Bass kernel for DenseDilatedKnnGraph (B=4, C=64, N=4096, k=9, dilation=2).

Algorithm
---------
reference: xt (B,N,C); dist(i,j) = |xi|^2 - 2<xi,xj> + |xj|^2; nn_idx = top-18
of -dist per row; output nn_idx[..., ::2] plus a center-index row
-> (2, B, N, 9) int32.

Per-row ordering of -dist equals the ordering of s_ij = 2<xi,xj> - |xj|^2.

Device (per core, SPMD over 8 cores; core = (batch, query-half)):
  - S via ONE fp8e4 DoubleRow matmul per 512-column chunk (0.5 cyc/col on the
    PE -- immune to the mid-pstate clock the PE is stuck at when its duty
    cycle is low).  K = 2 interleaved k-tiles of 128:
      ktile0 rows 0..63: qa x ca        rows 64..127: qb x ca
      ktile1 rows 0..63: qa x cb        rows 64..123: qb[0:60] x cb[0:60]
             rows 124..127: 1.0 x s1..s4 (4-level fp8 split of -|ca+cb|^2)
    where qa/qb = 2-level fp8e4 split of 2*x_q, ca/cb of x_cand.  S~ error
    sigma ~2.4e-2, max ~0.2 -- far below the value gaps at the family-select
    margin (measured worst needed-slot rank 19 of 2048).  Selection is
    approximate; the final ranking is exact (host, below).
  - PSUM evacuation fused with the select fold.  Only Act and DVE can read
    PSUM (DMA has no route, GpSimd has no port, DVE has ONE psum operand per
    instruction), so:
      Act: a32a/a32b = fp32 SBUF copies of PSUM pairs PA1 (cols 0:1024)
           and PA2 (cols 2048:3072)                       -- 2 insts/tile
      DVE: u16[:, :1024]  = max(PM2_psum, a32a)  -> fp16
           u16[:, 1024:]  = max(PM1_psum, a32b)  -> fp16
           (PM2 = cols 1024:2048, PM1 = cols 3072:4096)   -- 2 insts/tile
    Each DVE instruction evacuates 1024 PSUM values AND folds 2:1.
    Family slots (2048/row): slot c<1024: {c, 1024+c};
    slot 1024+u: {2048+u, 3072+u}.  No on-device top-k at all (the old
    kernel's max8/max_index full-row scans made DVE the 175us bottleneck).
  - DMA out u16 (2048 x 2048 fp16 per core).

Host: a family holding a true top-18 value ranks <= ~19 among the 2048 slot
values (at most 17 strictly larger values exist + fp8/fp16 noise), so the
top-40 slots per row are a guaranteed superset.  Gather their 80 candidate
columns, recompute s exactly (float64), stable top-18, dilate ::2.

History: 176.3us (max8/max_index full-row DVE scans) -> 104.0us (fp16 1-mm +
fold tree on DVE/Act) -> this version.
"""

import numpy as np
import ml_dtypes

import concourse.bacc as bacc
import concourse.mybir as mybir
import concourse.tile as tile
from concourse.bass_utils import run_bass_kernel_spmd

# Problem constants (hardcoded per harness contract).
B = 4
C = 64
N = 4096
K = 9
DILATION = 2
K_EFF = K * DILATION      # 18
P = 128                   # partitions / queries per tile
KM = 128                  # matmul contraction rows per k-tile
N_CORES = 8
QROWS = (B * N) // N_CORES          # 2048 query rows per core
N_TILES = QROWS // P                # 16 tiles per core
NSLOT = 2048              # family slots per row
KSEL = 40                 # slots resolved exactly on host
NQL = 60                  # qb channels kept in ktile1 (124..127 are bias)
F8 = ml_dtypes.float8_e4m3


def _build_program(n_tiles=N_TILES):
    nc = bacc.Bacc(
        "TRN2", target_bir_lowering=False, debug=False, enable_asserts=False
    )
    f32 = mybir.dt.float32
    f16 = mybir.dt.float16
    f8 = mybir.dt.float8e4
    amax = mybir.AluOpType.max
    dr = mybir.MatmulPerfMode.DoubleRow
    nq = n_tiles * P
    lhs = nc.dram_tensor("lhs", (KM, 2, nq), f8, kind="ExternalInput")
    rhs = nc.dram_tensor("rhs", (KM, 2, N), f8, kind="ExternalInput")
    u_out = nc.dram_tensor("u_out", (nq, NSLOT), f16, kind="ExternalOutput")
    lhs_ap, rhs_ap, u_ap = lhs.ap(), rhs.ap(), u_out.ap()

    with tile.TileContext(nc) as tc:
        with (
            tc.tile_pool(name="const", bufs=1) as cpool,
            tc.tile_pool(name="psA", bufs=1, space="PSUM") as ppoolA,
            tc.tile_pool(name="psB", bufs=1, space="PSUM") as ppoolB,
            tc.tile_pool(name="work", bufs=2) as wpool,
            tc.tile_pool(name="outp", bufs=4) as opool,
        ):
            # dependency-free warm-up matmuls during the input-DMA prologue
            prime = cpool.tile([KM, 2, 512], f8)
            nc.gpsimd.memset(prime[:, :, :], 0.0)
            warm = ppoolA.tile([P, 1024], f32, tag="pa1")
            for _ in range(12):
                nc.tensor.matmul(warm[:, :512], prime[:, :, :128],
                                 prime[:, :, :], start=True, stop=True,
                                 perf_mode=dr)

            # 3 big input DMAs on 2 queues (issue cost ~600ns each; many
            # small DMAs kept the Sync sequencer busy for 5+us at boot)
            rh_lo = cpool.tile([KM, 2, 2048], f8, name="rh_lo", tag="rh_lo")
            rh_hi = cpool.tile([KM, 2, 2048], f8, name="rh_hi", tag="rh_hi")
            lh = cpool.tile([KM, 2, nq], f8, name="lh", tag="lh")
            nc.sync.dma_start(rh_lo[:, :, :], rhs_ap[:, :, 0:2048])
            nc.scalar.dma_start(lh[:, :, :], lhs_ap[:, :, :])
            nc.sync.dma_start(rh_hi[:, :, :], rhs_ap[:, :, 2048:4096])

            for t in range(n_tiles):
                lsl = lh[:, :, t * P : (t + 1) * P]
                pa1 = ppoolA.tile([P, 1024], f32, tag="pa1")  # cols 0:1024
                pa2 = ppoolA.tile([P, 1024], f32, tag="pa2")  # cols 2048:3072
                pm1 = ppoolB.tile([P, 1024], f32, tag="pm1")  # cols 3072:4096
                pm2 = ppoolB.tile([P, 1024], f32, tag="pm2")  # cols 1024:2048
                for ps, src, c0 in (
                    (pa1, rh_lo, 0), (pm2, rh_lo, 1024),
                    (pa2, rh_hi, 0), (pm1, rh_hi, 1024),
                ):
                    for h in range(2):
                        nc.tensor.matmul(
                            ps[:, h * 512 : (h + 1) * 512],
                            lsl,
                            src[:, :, c0 + h * 512 : c0 + (h + 1) * 512],
                            start=True, stop=True, perf_mode=dr,
                        )

                a32a = wpool.tile([P, 1024], f32, tag="a32a")
                a32b = wpool.tile([P, 1024], f32, tag="a32b")
                nc.scalar.copy(a32a[:, :], pa1[:, :])
                nc.scalar.copy(a32b[:, :], pa2[:, :])

                u16 = opool.tile([P, NSLOT], f16, tag="u16")
                nc.vector.tensor_tensor(
                    u16[:, :1024], pm2[:, :], a32a[:, :], amax)
                nc.vector.tensor_tensor(
                    u16[:, 1024:], pm1[:, :], a32b[:, :], amax)

                nc.sync.dma_start(u_ap[t * P : (t + 1) * P, :], u16[:])
    nc.compile()
    return nc


def _split8(a):
    hi = a.astype(F8)
    lo = (a - hi.astype(np.float32)).astype(F8)
    return hi, lo


def _prep_core_inputs(X, core):
    """X: (B, N, C) fp32. Returns input map for one core."""
    b, h = divmod(core, N_CORES // B)
    Xb = X[b]
    ca, cb = _split8(Xb.T)                             # (C, N) each
    ceff = ca.astype(np.float32) + cb.astype(np.float32)
    csq = np.einsum("cn,cn->n", ceff, ceff)
    r = -csq
    svec = np.zeros((4, N), F8)
    for lv in range(4):
        svec[lv] = r.astype(F8)
        r = r - svec[lv].astype(np.float32)
    rhs = np.zeros((KM, 2, N), F8)
    rhs[:C, 0] = ca
    rhs[C:, 0] = ca
    rhs[:C, 1] = cb
    rhs[C : C + NQL, 1] = cb[:NQL]
    rhs[C + NQL :, 1] = svec

    Q = 2.0 * Xb[h * QROWS : (h + 1) * QROWS]          # (QROWS, C)
    qa, qb = _split8(Q.T)                              # (C, QROWS)
    lhs = np.zeros((KM, 2, QROWS), F8)
    lhs[:C, 0] = qa
    lhs[C:, 0] = qb
    lhs[:C, 1] = qa
    lhs[C : C + NQL, 1] = qb[:NQL]
    lhs[C + NQL :, 1] = 1.0
    return {"lhs": lhs, "rhs": rhs}


def _slot_map():
    """slot -> 2 candidate columns (see fold structure in the header)."""
    m = np.empty((NSLOT, 2), np.int64)
    c = np.arange(1024)
    m[:1024, 0] = c
    m[:1024, 1] = 1024 + c
    m[1024:, 0] = 2048 + c
    m[1024:, 1] = 3072 + c
    return m


_NC_CACHE = {}
_SLOT_MAP = _slot_map()


def kernel(x: np.ndarray) -> np.ndarray:
    x = np.asarray(x)
    assert x.shape == (B, C, N, 1), x.shape
    X = np.ascontiguousarray(np.transpose(x[..., 0], (0, 2, 1)))  # (B, N, C)

    if N_TILES not in _NC_CACHE:
        _NC_CACHE[N_TILES] = _build_program(N_TILES)
    nc = _NC_CACHE[N_TILES]

    in_maps = [_prep_core_inputs(X, c) for c in range(N_CORES)]
    res = run_bass_kernel_spmd(nc, in_maps, core_ids=list(range(N_CORES)))

    nn_idx = np.empty((B, N, K_EFF), np.int64)
    for core in range(N_CORES):
        b, h = divmod(core, N_CORES // B)
        U = np.asarray(res.results[core]["u_out"])        # (QROWS, NSLOT) f16
        sel = np.argpartition(-U, KSEL, axis=1)[:, :KSEL]
        cand = _SLOT_MAP[sel].reshape(QROWS, 2 * KSEL)
        cand.sort(axis=1)
        Xb = X[b].astype(np.float64)
        Q = 2.0 * Xb[h * QROWS : (h + 1) * QROWS]
        xsq = np.einsum("nc,nc->n", Xb, Xb)
        G = Xb[cand]                                      # (QROWS, 2K, C)
        s_c = np.einsum("rkc,rc->rk", G, Q) - xsq[cand]
        oo = np.argsort(-s_c, axis=1, kind="stable")[:, :K_EFF]
        nn_idx[b, h * QROWS : (h + 1) * QROWS] = np.take_along_axis(
            cand, oo, axis=1
        )

    nn_dil = nn_idx[:, :, ::DILATION]                     # (B, N, 9)
    center = np.broadcast_to(np.arange(N)[None, :, None], nn_dil.shape)
    out = np.stack((nn_dil, center), axis=0).astype(np.int32)
    return out
